# revision 29
# baseline (speedup 1.0000x reference)
"""MultiHeadAttention Trainium2 Bass kernel, 8-core SPMD.

Problem: B=4, S=2048, E=2048, H=16, Dh=128; reshape-based (not transposed)
head split:  q = (x@Wq).reshape(B,H,S,Dh) etc., softmax over the QUERY axis,
out = attn.reshape(B,S,E).

Key structure: flattening (B,S) rows, row-block gp (128 rows) of x@W is
exactly head pair gp=(b,h): Qh = Y[128gp:128gp+128,:].reshape(2048,128).
Each of the 8 cores handles 8 consecutive pairs -> core c gets contiguous
x rows [1024c:1024c+1024) and produces the same output rows. No collectives.

Internal q/k index permutation (order-free since softmax reduces over q):
f = j*128 + s  <->  q = 16s + j. With that permutation QT/KT [d, f] is the
j-th 128-col block of (Xblk @ W)^T and the Vh block kj is the e-block kj of
Yv = Xblk@Wv in natural [s, e] layout.

Schedule (v2): single pass over the weights (each W tile streamed once for
all 8 pairs), full xT resident in SBUF, QT/KT spilled to DRAM, YV resident
in bf16.  Attention phase is software-pipelined pair-to-pair: scores+softmax
of pair p+1 interleave with the attn matmuls of pair p so the PE never
starves (keeps HAM at K=8/8).  Z-sum on GpSimd, exp on Scalar, max on DVE.
SCALE is folded into Wq on the host.

Dtypes: fp32r for projections + scores; bf16 softmax weights + attn;
fp32 PSUM/softmax stats.
"""

import numpy as np
from contextlib import ExitStack

import concourse.bass as bass
import concourse.tile as tile
from concourse import bacc, mybir
from concourse.bass import ds, ts
from concourse.bass_utils import run_bass_kernel_spmd
from concourse.masks import make_identity

F32 = mybir.dt.float32
F32R = mybir.dt.float32r
BF16 = mybir.dt.bfloat16
F16 = mybir.dt.float16
AX = mybir.AxisListType.X
EXP = mybir.ActivationFunctionType.Exp

P = 128
NPAIR = 8          # (b,h) pairs per core
NJ = 16            # 128-blocks in E / contraction
SCALE = 1.0 / np.sqrt(128.0)

_cache = {}


def _emit(nc, tc, ctx, xl, wq, wk, wv, idr, out):
    sb = ctx.enter_context
    # ---- persistent pools ----
    pW = sb(tc.tile_pool(name="pw", bufs=3))          # QK weight tiles, 1MB each
    pSTG = sb(tc.tile_pool(name="pstg", bufs=2))      # spill staging
    pYV = sb(tc.tile_pool(name="pyv", bufs=1))        # YV bf16 per pair (8 tags)
    pST = sb(tc.tile_pool(name="pst", bufs=8))        # small stats
    pCONST = sb(tc.tile_pool(name="pconst", bufs=1))
    psSC = sb(tc.tile_pool(name="pssc", bufs=3, space="PSUM"))   # [128,1024] x3 = 6 banks
    psAT = sb(tc.tile_pool(name="psat", bufs=1, space="PSUM"))   # [128,1024] x1 = 2 banks
    dram = sb(tc.tile_pool(name="dram", bufs=1, space="DRAM"))

    qsp = dram.tile([P, NPAIR, NJ, P], F32R, tag="qsp")
    ksp = dram.tile([P, NPAIR, NJ, P], F32R, tag="ksp")

    ident = pCONST.tile([P, P], F32, tag="ident")
    make_identity(nc, ident[:])
    identr = pCONST.tile([P, P], F32R, tag="identr")
    nc.sync.dma_start(identr[:], idr)

    yv_tiles = {}

    # ================= phases A + B (projections) =================
    with tc.tile_pool(name="pxt", bufs=1) as pXT, \
         tc.tile_pool(name="pwv", bufs=2) as pWV, \
         tc.tile_pool(name="pin", bufs=2) as pIN:
        # ---- A: transpose all of x into xT [P, kb, pair, s] ----
        xT = pXT.tile([P, NJ, NPAIR, P], F32R, tag="xt")
        for pi in range(NPAIR):
            xt = pIN.tile([P, NJ * P], F32R, tag="xin")
            nc.sync.dma_start(xt[:], xl[ds(pi * P, P), :])
            for jj in range(2):
                pt = psSC.tile([P, 1024], F32, tag="sc")
                for i in range(8):
                    j = jj * 8 + i
                    nc.tensor.transpose(
                        pt[:, ds(i * P, P)].bitcast(F32R), xt[:, ds(j * P, P)],
                        identr[:],
                    )
                nc.vector.tensor_copy(
                    xT[:, ts(jj, 8), pi, :],
                    pt[:].rearrange("p (a b) -> p a b", a=8),
                )

        # ---- B-QK: stream each weight tile once over all 8 pairs ----
        for wd, sp in ((wq, qsp), (wk, ksp)):
            for j in range(NJ):
                wt = pW.tile([P, NJ, P], F32R, tag="w")
                nc.sync.dma_start(wt[:], wd[j])
                ps = psSC.tile([P, 1024], F32, tag="sc")
                for kb in range(NJ):
                    for h in range(2):
                        nc.tensor.matmul(
                            ps[:, ds(h * 512, 512)], wt[:, kb],
                            xT[:, kb, ds(h * 4, 4), :],
                            start=(kb == 0), stop=(kb == NJ - 1),
                        )
                stg = pSTG.tile([P, NPAIR, P], F32R, tag="stg")
                nc.vector.tensor_copy(
                    stg[:], ps[:].rearrange("p (g s) -> p g s", g=NPAIR)
                )
                nc.sync.dma_start(sp[:, :, j, :], stg[:])

        # ---- B-V: Wv in 256-col strips, YV resident bf16 ----
        for pi in range(NPAIR):
            yv_tiles[pi] = pYV.tile([P, NJ * P], BF16, tag=f"yv{pi}",
                                    name=f"yv{pi}")
        for sc_i in range(8):
            wvt = pWV.tile([P, NJ, 256], F32R, tag="wv")
            nc.sync.dma_start(wvt[:], wv[sc_i])
            for pi in range(NPAIR):
                ps = psSC.tile([P, 1024], F32, tag="sc")
                for kb in range(NJ):
                    nc.tensor.matmul(
                        ps[:, :256], xT[:, kb, pi], wvt[:, kb],
                        start=(kb == 0), stop=(kb == NJ - 1),
                    )
                nc.vector.tensor_copy(
                    yv_tiles[pi][:, ds(sc_i * 256, 256)], ps[:, :256]
                )

    # ================= phase C (attention), software-pipelined =================
    # Flat stream over tiles T = pair*16 + kj. Per emission slot T:
    #   - Z/recip/vs for tile T-2  (soft complete)
    #   - attn MMs for pair q=T//16-1 (seg-accumulated in 2-bank psum)
    #   - out-transposes for pair r=T//16-2 (borrow the psAT rotation)
    #   - scores + fused-neg-max for tile T
    #   - exp for tile T-1
    NT = NPAIR * NJ
    with tc.tile_pool(name="pqk", bufs=2) as pQK, \
         tc.tile_pool(name="pacc", bufs=2) as pACC, \
         tc.tile_pool(name="psoft", bufs=20) as pSOFT, \
         tc.tile_pool(name="pvs", bufs=20) as pVS:

        qts, kts = {}, {}
        pss = {}     # T -> (ps_h0, ps_h1)
        nmd = {}     # T -> negated max [P,1]
        lsd = {}     # T -> h0 exp accumulator [P,1]
        softd = {}   # T -> soft tile
        vsd = {}     # T -> scaled V tile
        accs = {}    # pair -> acc tile
        paseg = {}   # pair -> current attn psum segment tile
        pending_seg = []   # deferred seg psum->acc finalizations

        def c1_start(pair):
            qt = pQK.tile([P, NJ, P], F32R, tag="qt")
            nc.sync.dma_start(qt[:], qsp[:, pair])
            kt = pQK.tile([P, NJ, P], F32R, tag="kt")
            nc.sync.dma_start(kt[:], ksp[:, pair])
            qts[pair], kts[pair] = qt, kt

        def scores_chunk(T, h):
            pair, kj = divmod(T, NJ)
            qt, kt = qts[pair], kts[pair]
            ps = psSC.tile([P, 1024], F32, tag="sc")
            for c in range(2):
                nc.tensor.matmul(
                    ps[:, ds(c * 512, 512)], kt[:, kj],
                    qt[:, ts(h * 2 + c, 4), :], start=True, stop=True,
                )
            pss.setdefault(T, []).append(ps)

        def max_min(T):
            nms = []
            for h in range(2):
                nm = pST.tile([P, 1], F32, tag="nm")
                nc.vector.reduce_max(nm[:], pss[T][h][:], axis=AX,
                                     negate=True)
                nms.append(nm)
            ng = pST.tile([P, 1], F32, tag="ng")
            nc.vector.tensor_tensor(ng[:], nms[0][:], nms[1][:],
                                    mybir.AluOpType.min)
            nmd[T] = ng

        def expstep(T):
            soft = pSOFT.tile([P, 2048], BF16, tag="soft")
            ls = pST.tile([P, 2], F32, tag="ls")
            for h in range(2):
                nc.scalar.activation(soft[:, ds(h * 1024, 1024)],
                                     pss[T][h][:], EXP, bias=nmd[T][:],
                                     accum_out=ls[:, ds(h, 1)])
            del pss[T]
            softd[T] = soft
            lsd[T] = ls

        def zrv(T):
            pair, kj = divmod(T, NJ)
            zt = pST.tile([P, 1], F32, tag="zt")
            ls = lsd.pop(T)
            nc.vector.tensor_tensor(zt[:], ls[:, ds(0, 1)], ls[:, ds(1, 1)],
                                    mybir.AluOpType.add)
            rcp = pST.tile([P, 1], F32, tag="rcp")
            nc.vector.reciprocal(rcp[:], zt[:])
            vs = pVS.tile([P, P], BF16, tag="vs")
            nc.vector.tensor_scalar_mul(vs[:], yv_tiles[pair][:, ts(kj, P)],
                                        rcp[:])
            vsd[T] = vs

        def attn_half(q, w, half):
            # segments: s=w//4 -> (g,h); 8 kj per segment, 2 kj per slot
            s, wi = divmod(w, 4)
            g, h = divmod(s, 2)
            if wi == 0 and half == 0:
                paseg[q] = psAT.tile([P, 1024], F32, tag="at", name="pa")
            pa = paseg[q]
            kj = 8 * g + 2 * wi + half
            for c in range(2):
                nc.tensor.matmul(
                    pa[:, ds(c * 512, 512)], vsd[q * NJ + kj][:],
                    softd[q * NJ + kj][:, ds(h * 1024 + c * 512, 512)],
                    start=(kj == 8 * g), stop=(kj == 8 * g + 7),
                    skip_group_check=True,
                )
            if wi == 3 and half == 1:
                pending_seg.append((q, g, h, pa))

        def seg_finalize():
            # deferred psum->acc copies/adds, on DVE at slot end so they
            # never block the exp chain on the Scalar queue
            while pending_seg:
                q, g, h, pa = pending_seg.pop(0)
                if g == 0:
                    accs[q] = accs.get(q) or pACC.tile([P, NJ * P], F32,
                                                       tag="acc", name="acc")
                    nc.vector.tensor_copy(accs[q][:, ds(h * 1024, 1024)],
                                          pa[:])
                else:
                    nc.vector.tensor_add(accs[q][:, ds(h * 1024, 1024)],
                                         accs[q][:, ds(h * 1024, 1024)],
                                         pa[:])

        def tail_step(r, w):
            # 4 out-transposes + 1 writeback per boundary slot, DMA at w==15
            if w % 4 != 3:
                return
            jj = w // 4
            acc = accs[r]
            pt = psAT.tile([P, 1024], F32, tag="at")
            for i in range(4):
                c = jj * 4 + i
                nc.tensor.transpose(pt[:, ds(i * P, P)], acc[:, ds(c * P, P)],
                                    ident[:])
            nc.vector.tensor_copy(acc[:, ds(jj * 512, 512)], pt[:, :512])
            if w == 15:
                nc.sync.dma_start(out[ds(r * P, P), :], acc[:])
                del accs[r]

        c1_start(0)
        for T in range(NT + 2 * NJ):
            q, w = T // NJ - 1, T % NJ
            r = q - 1
            if 2 <= T < NT + 2:
                zrv(T - 2)
            if 1 <= T < NT + 1:
                expstep(T - 1)
            # PE stream: both score chunks first (each issues as soon as its
            # psum gate frees), then attn MMs fill the reduce/min wait window
            if T < NT:
                scores_chunk(T, 0)
                scores_chunk(T, 1)
            if 0 <= q < NPAIR:
                attn_half(q, w, 0)
                attn_half(q, w, 1)
            if T < NT:
                max_min(T)
            seg_finalize()
            if 0 <= r < NPAIR:
                tail_step(r, w)
            if w == 8 and 0 <= q + 2 < NPAIR:
                c1_start(q + 2)
            # free consumed tiles
            vsd.pop(T - 2 * NJ, None)
            softd.pop(T - 2 * NJ, None)


def build(compile=True):
    key = ("nc", compile)
    if key in _cache:
        return _cache[key]
    nc = bacc.Bacc("TRN2", target_bir_lowering=False, debug=False)
    xl = nc.dram_tensor("xl", [NPAIR * P, 2048], F32R, kind="ExternalInput").ap()
    wq = nc.dram_tensor("wq", [NJ, P, NJ, P], F32R, kind="ExternalInput").ap()
    wk = nc.dram_tensor("wk", [NJ, P, NJ, P], F32R, kind="ExternalInput").ap()
    wv = nc.dram_tensor("wv", [8, P, NJ, 256], F32R, kind="ExternalInput").ap()
    idr = nc.dram_tensor("idr", [P, P], F32R, kind="ExternalInput").ap()
    out = nc.dram_tensor("out", [NPAIR * P, 2048], F32, kind="ExternalOutput").ap()
    with tile.TileContext(nc) as tc:
        with ExitStack() as ctx:
            _emit(nc, tc, ctx, xl, wq, wk, wv, idr, out)
    if compile:
        nc.compile()
    _cache[key] = nc
    return nc


def kernel(x, w_query, w_key, w_value, _want_trace=False):
    x = np.ascontiguousarray(np.asarray(x, np.float32))
    wq = np.ascontiguousarray(np.asarray(w_query, np.float32))
    wk = np.ascontiguousarray(np.asarray(w_key, np.float32))
    wv = np.ascontiguousarray(np.asarray(w_value, np.float32))
    B, S, E = x.shape
    xf = x.reshape(B * S, E)
    nc = build()
    rows = NPAIR * P
    # fold the 1/sqrt(Dh) score scale into Wq on the host
    wq_t = np.ascontiguousarray(
        (wq * np.float32(SCALE)).reshape(NJ, P, NJ, P).transpose(2, 1, 0, 3))
    wk_t = np.ascontiguousarray(wk.reshape(NJ, P, NJ, P).transpose(2, 1, 0, 3))
    wv_t = np.ascontiguousarray(wv.reshape(NJ, P, 8, 256).transpose(2, 1, 0, 3))
    eye = np.eye(P, dtype=np.float32)
    in_maps = [
        dict(xl=np.ascontiguousarray(xf[c * rows:(c + 1) * rows]),
             wq=wq_t, wk=wk_t, wv=wv_t, idr=eye)
        for c in range(8)
    ]
    res = run_bass_kernel_spmd(nc, in_maps, core_ids=list(range(8)),
                               trace=_want_trace)
    outf = np.concatenate([r["out"] for r in res.results], axis=0)
    if _want_trace:
        kernel.last_result = res
    return outf.reshape(B, S, E)


# revision 31
# speedup vs baseline: 1.0082x; 1.0082x over previous
"""MultiHeadAttention Trainium2 Bass kernel, 8-core SPMD.

Problem: B=4, S=2048, E=2048, H=16, Dh=128; reshape-based (not transposed)
head split:  q = (x@Wq).reshape(B,H,S,Dh) etc., softmax over the QUERY axis,
out = attn.reshape(B,S,E).

Key structure: flattening (B,S) rows, row-block gp (128 rows) of x@W is
exactly head pair gp=(b,h): Qh = Y[128gp:128gp+128,:].reshape(2048,128).
Each of the 8 cores handles 8 consecutive pairs -> core c gets contiguous
x rows [1024c:1024c+1024) and produces the same output rows. No collectives.

Internal q/k index permutation (order-free since softmax reduces over q):
f = j*128 + s  <->  q = 16s + j. With that permutation QT/KT [d, f] is the
j-th 128-col block of (Xblk @ W)^T and the Vh block kj is the e-block kj of
Yv = Xblk@Wv in natural [s, e] layout.

Schedule (v2): single pass over the weights (each W tile streamed once for
all 8 pairs), full xT resident in SBUF, QT/KT spilled to DRAM, YV resident
in bf16.  Attention phase is software-pipelined pair-to-pair: scores+softmax
of pair p+1 interleave with the attn matmuls of pair p so the PE never
starves (keeps HAM at K=8/8).  Z-sum on GpSimd, exp on Scalar, max on DVE.
SCALE is folded into Wq on the host.

Dtypes: fp32r for projections + scores; bf16 softmax weights + attn;
fp32 PSUM/softmax stats.
"""

import numpy as np
from contextlib import ExitStack

import concourse.bass as bass
import concourse.tile as tile
from concourse import bacc, mybir
from concourse.bass import ds, ts
from concourse.bass_utils import run_bass_kernel_spmd
from concourse.masks import make_identity

F32 = mybir.dt.float32
F32R = mybir.dt.float32r
BF16 = mybir.dt.bfloat16
F16 = mybir.dt.float16
AX = mybir.AxisListType.X
EXP = mybir.ActivationFunctionType.Exp

P = 128
NPAIR = 8          # (b,h) pairs per core
NJ = 16            # 128-blocks in E / contraction
SCALE = 1.0 / np.sqrt(128.0)

_cache = {}


def _emit(nc, tc, ctx, xl, wq, wk, wv, idr, out):
    sb = ctx.enter_context
    # ---- persistent pools ----
    pW = sb(tc.tile_pool(name="pw", bufs=3))          # QK weight tiles, 1MB each
    pSTG = sb(tc.tile_pool(name="pstg", bufs=2))      # spill staging
    pYV = sb(tc.tile_pool(name="pyv", bufs=1))        # YV bf16 per pair (8 tags)
    pST = sb(tc.tile_pool(name="pst", bufs=8))        # small stats
    pCONST = sb(tc.tile_pool(name="pconst", bufs=1))
    psSC = sb(tc.tile_pool(name="pssc", bufs=3, space="PSUM"))   # [128,1024] x3 = 6 banks
    psAT = sb(tc.tile_pool(name="psat", bufs=1, space="PSUM"))   # [128,1024] x1 = 2 banks
    dram = sb(tc.tile_pool(name="dram", bufs=1, space="DRAM"))

    qsp = dram.tile([P, NPAIR, NJ, P], F32R, tag="qsp")
    ksp = dram.tile([P, NPAIR, NJ, P], F32R, tag="ksp")

    ident = pCONST.tile([P, P], F32, tag="ident")
    make_identity(nc, ident[:])
    identr = pCONST.tile([P, P], F32R, tag="identr")
    nc.sync.dma_start(identr[:], idr)

    yv_tiles = {}

    # ================= phases A + B (projections) =================
    with tc.tile_pool(name="pxt", bufs=1) as pXT, \
         tc.tile_pool(name="pwv", bufs=2) as pWV, \
         tc.tile_pool(name="pin", bufs=2) as pIN:
        # ---- A: transpose all of x into xT [P, kb, pair, s] ----
        xT = pXT.tile([P, NJ, NPAIR, P], F32R, tag="xt")
        for pi in range(NPAIR):
            xt = pIN.tile([P, NJ * P], F32R, tag="xin")
            nc.sync.dma_start(xt[:], xl[ds(pi * P, P), :])
            for jj in range(2):
                pt = psSC.tile([P, 1024], F32, tag="sc")
                for i in range(8):
                    j = jj * 8 + i
                    nc.tensor.transpose(
                        pt[:, ds(i * P, P)].bitcast(F32R), xt[:, ds(j * P, P)],
                        identr[:],
                    )
                nc.vector.tensor_copy(
                    xT[:, ts(jj, 8), pi, :],
                    pt[:].rearrange("p (a b) -> p a b", a=8),
                )

        # ---- B-QK: stream each weight tile once over all 8 pairs ----
        for wd, sp in ((wq, qsp), (wk, ksp)):
            for j in range(NJ):
                wt = pW.tile([P, NJ, P], F32R, tag="w")
                nc.sync.dma_start(wt[:], wd[j])
                ps = psSC.tile([P, 1024], F32, tag="sc")
                for kb in range(NJ):
                    for h in range(2):
                        nc.tensor.matmul(
                            ps[:, ds(h * 512, 512)], wt[:, kb],
                            xT[:, kb, ds(h * 4, 4), :],
                            start=(kb == 0), stop=(kb == NJ - 1),
                        )
                stg = pSTG.tile([P, NPAIR, P], F32R, tag="stg")
                nc.vector.tensor_copy(
                    stg[:], ps[:].rearrange("p (g s) -> p g s", g=NPAIR)
                )
                nc.sync.dma_start(sp[:, :, j, :], stg[:])

        # ---- B-V: Wv in 256-col strips, YV resident bf16 ----
        for pi in range(NPAIR):
            yv_tiles[pi] = pYV.tile([P, NJ * P], BF16, tag=f"yv{pi}",
                                    name=f"yv{pi}")
        for sc_i in range(8):
            wvt = pWV.tile([P, NJ, 256], F32R, tag="wv")
            nc.sync.dma_start(wvt[:], wv[sc_i])
            for pi in range(NPAIR):
                ps = psSC.tile([P, 1024], F32, tag="sc")
                for kb in range(NJ):
                    nc.tensor.matmul(
                        ps[:, :256], xT[:, kb, pi], wvt[:, kb],
                        start=(kb == 0), stop=(kb == NJ - 1),
                    )
                nc.vector.tensor_copy(
                    yv_tiles[pi][:, ds(sc_i * 256, 256)], ps[:, :256]
                )

    # ================= phase C (attention), software-pipelined =================
    # Flat stream over tiles T = pair*16 + kj. Per emission slot T:
    #   - Z/recip/vs for tile T-2  (soft complete)
    #   - attn MMs for pair q=T//16-1 (seg-accumulated in 2-bank psum)
    #   - out-transposes for pair r=T//16-2 (borrow the psAT rotation)
    #   - scores + fused-neg-max for tile T
    #   - exp for tile T-1
    NT = NPAIR * NJ
    with tc.tile_pool(name="pqk", bufs=2) as pQK, \
         tc.tile_pool(name="pacc", bufs=2) as pACC, \
         tc.tile_pool(name="psoft", bufs=21) as pSOFT, \
         tc.tile_pool(name="pvs", bufs=21) as pVS:

        qts, kts = {}, {}
        pss = {}     # T -> (ps_h0, ps_h1)
        nmd = {}     # T -> negated max [P,1]
        lsd = {}     # T -> h0 exp accumulator [P,1]
        softd = {}   # T -> soft tile
        vsd = {}     # T -> scaled V tile
        accs = {}    # pair -> acc tile
        paseg = {}   # pair -> current attn psum segment tile
        pending_seg = []   # deferred seg psum->acc finalizations

        def c1_start(pair):
            qt = pQK.tile([P, NJ, P], F32R, tag="qt")
            nc.sync.dma_start(qt[:], qsp[:, pair])
            kt = pQK.tile([P, NJ, P], F32R, tag="kt")
            nc.sync.dma_start(kt[:], ksp[:, pair])
            qts[pair], kts[pair] = qt, kt

        def scores_chunk(T, h):
            pair, kj = divmod(T, NJ)
            qt, kt = qts[pair], kts[pair]
            ps = psSC.tile([P, 1024], F32, tag="sc")
            for c in range(2):
                nc.tensor.matmul(
                    ps[:, ds(c * 512, 512)], kt[:, kj],
                    qt[:, ts(h * 2 + c, 4), :], start=True, stop=True,
                )
            pss.setdefault(T, []).append(ps)

        def max_min(T):
            nms = []
            for h in range(2):
                nm = pST.tile([P, 1], F32, tag="nm")
                nc.vector.reduce_max(nm[:], pss[T][h][:], axis=AX,
                                     negate=True)
                nms.append(nm)
            ng = pST.tile([P, 1], F32, tag="ng")
            nc.vector.tensor_tensor(ng[:], nms[0][:], nms[1][:],
                                    mybir.AluOpType.min)
            nmd[T] = ng

        def expstep(T):
            soft = pSOFT.tile([P, 2048], BF16, tag="soft")
            ls = pST.tile([P, 2], F32, tag="ls")
            for h in range(2):
                nc.scalar.activation(soft[:, ds(h * 1024, 1024)],
                                     pss[T][h][:], EXP, bias=nmd[T][:],
                                     accum_out=ls[:, ds(h, 1)])
            del pss[T]
            softd[T] = soft
            lsd[T] = ls

        def zrv(T):
            pair, kj = divmod(T, NJ)
            zt = pST.tile([P, 1], F32, tag="zt")
            ls = lsd.pop(T)
            nc.vector.tensor_tensor(zt[:], ls[:, ds(0, 1)], ls[:, ds(1, 1)],
                                    mybir.AluOpType.add)
            rcp = pST.tile([P, 1], F32, tag="rcp")
            nc.vector.reciprocal(rcp[:], zt[:])
            vs = pVS.tile([P, P], BF16, tag="vs")
            nc.vector.tensor_scalar_mul(vs[:], yv_tiles[pair][:, ts(kj, P)],
                                        rcp[:])
            vsd[T] = vs

        def attn_half(q, w, half):
            # segments: s=w//4 -> (g,h); 8 kj per segment, 2 kj per slot
            s, wi = divmod(w, 4)
            g, h = divmod(s, 2)
            if wi == 0 and half == 0:
                paseg[q] = psAT.tile([P, 1024], F32, tag="at", name="pa")
            pa = paseg[q]
            kj = 8 * g + 2 * wi + half
            for c in range(2):
                nc.tensor.matmul(
                    pa[:, ds(c * 512, 512)], vsd[q * NJ + kj][:],
                    softd[q * NJ + kj][:, ds(h * 1024 + c * 512, 512)],
                    start=(kj == 8 * g), stop=(kj == 8 * g + 7),
                    skip_group_check=True,
                )
            if wi == 3 and half == 1:
                pending_seg.append((q, g, h, pa))

        def seg_finalize():
            # deferred psum->acc copies/adds, on DVE at slot end so they
            # never block the exp chain on the Scalar queue
            while pending_seg:
                q, g, h, pa = pending_seg.pop(0)
                if g == 0:
                    accs[q] = accs.get(q) or pACC.tile([P, NJ * P], F32,
                                                       tag="acc", name="acc")
                    nc.vector.tensor_copy(accs[q][:, ds(h * 1024, 1024)],
                                          pa[:])
                else:
                    nc.vector.tensor_add(accs[q][:, ds(h * 1024, 1024)],
                                         accs[q][:, ds(h * 1024, 1024)],
                                         pa[:])

        def tail_step(r, w):
            # 4 out-transposes + 1 writeback per boundary slot, DMA at w==15
            if w % 4 != 3:
                return
            jj = w // 4
            acc = accs[r]
            pt = psAT.tile([P, 1024], F32, tag="at")
            for i in range(4):
                c = jj * 4 + i
                nc.tensor.transpose(pt[:, ds(i * P, P)], acc[:, ds(c * P, P)],
                                    ident[:])
            nc.vector.tensor_copy(acc[:, ds(jj * 512, 512)], pt[:, :512])
            if w == 15:
                nc.sync.dma_start(out[ds(r * P, P), :], acc[:])
                del accs[r]

        c1_start(0)
        for T in range(NT + 2 * NJ):
            q, w = T // NJ - 1, T % NJ
            r = q - 1
            if 2 <= T < NT + 2:
                zrv(T - 2)
            if 1 <= T < NT + 1:
                expstep(T - 1)
            # PE stream: both score chunks first (each issues as soon as its
            # psum gate frees), then attn MMs fill the reduce/min wait window
            if T < NT:
                scores_chunk(T, 0)
                scores_chunk(T, 1)
            if 0 <= q < NPAIR:
                attn_half(q, w, 0)
                attn_half(q, w, 1)
            if T < NT:
                max_min(T)
            seg_finalize()
            if 0 <= r < NPAIR:
                tail_step(r, w)
            if w == 8 and 0 <= q + 2 < NPAIR:
                c1_start(q + 2)
            # free consumed tiles
            vsd.pop(T - 2 * NJ, None)
            softd.pop(T - 2 * NJ, None)


def build(compile=True):
    key = ("nc", compile)
    if key in _cache:
        return _cache[key]
    nc = bacc.Bacc("TRN2", target_bir_lowering=False, debug=False)
    xl = nc.dram_tensor("xl", [NPAIR * P, 2048], F32R, kind="ExternalInput").ap()
    wq = nc.dram_tensor("wq", [NJ, P, NJ, P], F32R, kind="ExternalInput").ap()
    wk = nc.dram_tensor("wk", [NJ, P, NJ, P], F32R, kind="ExternalInput").ap()
    wv = nc.dram_tensor("wv", [8, P, NJ, 256], F32R, kind="ExternalInput").ap()
    idr = nc.dram_tensor("idr", [P, P], F32R, kind="ExternalInput").ap()
    out = nc.dram_tensor("out", [NPAIR * P, 2048], F32, kind="ExternalOutput").ap()
    with tile.TileContext(nc) as tc:
        with ExitStack() as ctx:
            _emit(nc, tc, ctx, xl, wq, wk, wv, idr, out)
    if compile:
        nc.compile()
    _cache[key] = nc
    return nc


def kernel(x, w_query, w_key, w_value, _want_trace=False):
    x = np.ascontiguousarray(np.asarray(x, np.float32))
    wq = np.ascontiguousarray(np.asarray(w_query, np.float32))
    wk = np.ascontiguousarray(np.asarray(w_key, np.float32))
    wv = np.ascontiguousarray(np.asarray(w_value, np.float32))
    B, S, E = x.shape
    xf = x.reshape(B * S, E)
    nc = build()
    rows = NPAIR * P
    # fold the 1/sqrt(Dh) score scale into Wq on the host
    wq_t = np.ascontiguousarray(
        (wq * np.float32(SCALE)).reshape(NJ, P, NJ, P).transpose(2, 1, 0, 3))
    wk_t = np.ascontiguousarray(wk.reshape(NJ, P, NJ, P).transpose(2, 1, 0, 3))
    wv_t = np.ascontiguousarray(wv.reshape(NJ, P, 8, 256).transpose(2, 1, 0, 3))
    eye = np.eye(P, dtype=np.float32)
    in_maps = [
        dict(xl=np.ascontiguousarray(xf[c * rows:(c + 1) * rows]),
             wq=wq_t, wk=wk_t, wv=wv_t, idr=eye)
        for c in range(8)
    ]
    res = run_bass_kernel_spmd(nc, in_maps, core_ids=list(range(8)),
                               trace=_want_trace)
    outf = np.concatenate([r["out"] for r in res.results], axis=0)
    if _want_trace:
        kernel.last_result = res
    return outf.reshape(B, S, E)
